# revision 15
# baseline (speedup 1.0000x reference)
"""MoE layer (T=2048, D=1024, H=4096, E=8, top-2) on 8 trn2 NeuronCores.

Expert-parallel: core c holds expert c's weights. Every core computes the gate
for its 256-token slice, an AllGather replicates the logits, each core then
recomputes the full top-2 routing locally (identical on all cores), gathers
its expert's tokens (capacity C=576) via indirect DMA, runs the FFN in fp32r,
and sends gate-weighted output rows to the token-owner cores with one
AllToAll (bf16, 96 rows per (expert, owner) pair, t_local carried in-band).
Owners scatter-add the received rows into their 256-token output slice.

Self-contained: `kernel(**inputs) -> np.ndarray` takes full inputs, returns
the full [1, 2048, 1024] output.
"""
import os
import numpy as np
from contextlib import ExitStack

import concourse.bass as bass
import concourse.bacc as bacc
import concourse.mybir as mybir
import concourse.tile as tile
from concourse.bass_utils import run_bass_kernel_spmd

F32 = mybir.dt.float32
F32R = mybir.dt.float32r
BF16 = mybir.dt.bfloat16
I32 = mybir.dt.int32
AF = mybir.ActivationFunctionType
OP = mybir.AluOpType

T, D, H, E = 2048, 1024, 4096, 8
TS = T // E          # tokens per owner slice = 256
NS = T // 128        # 16 token columns (t = s*128 + p)
C = 576              # per-expert compute capacity (actual max count 551)
NC_CHUNKS = (C + 127) // 128          # 5 slot chunks (last is 64 wide)
CH = C // 2          # 288, MM1 free-dim half (>=256 keeps fp32r at full rate)
PAIRC = 96           # capacity per (expert, owner) pair (actual max 80)
AW = 1040            # a2a row: 1024 y + 1 t_local + 15 pad (bf16)
AROWS = E * PAIRC    # 768
BIGF = 1.0e30
OOB = 4096.0

LAST_EXEC_NS = [None]
PHASE = int(os.environ.get("BASSMOE_PHASE", "8"))


def _build_nc(trace_names=False):
    nc = bacc.Bacc(None, num_devices=E)
    din = {}
    for name, shape, dt in [
        ("x_full", [T, D], F32),
        ("x_slice", [TS, D], F32),
        ("gate_w", [D, E], F32),
        ("gate_b_rep", [128, E], F32),
        ("w1_e", [D, H], F32),
        ("b1_e", [128, H // 128], F32),
        ("w2_e", [H, D], F32),
        ("b2_rep", [128, D], F32),
        ("onehot_e", [128, E], F32),
        ("ident", [128, 128], F32),
        ("sut", [128, 128], F32),
        ("ones_col", [128, 1], F32),
        ("ones_row", [1, 128], F32),
        ("iota_t", [128, NS], F32),
        ("tloc", [128, NS], F32),
        ("own96", [128, NS], F32),
        ("iota_c", [128, C], F32),
    ]:
        din[name] = nc.dram_tensor(name, shape, dt, kind="ExternalInput")
    out_ext = nc.dram_tensor("out", [TS, D], F32, kind="ExternalOutput")

    with ExitStack() as ctx:
        tc = ctx.enter_context(tile.TileContext(nc))
        sb = ctx.enter_context(tc.tile_pool(name="sb", bufs=1))
        dram = ctx.enter_context(tc.tile_pool(name="dram", bufs=1, space="DRAM"))

        # ---------------- persistent small tiles ----------------
        sut_sb = sb.tile([128, 128], F32)
        nc.sync.dma_start(sut_sb[:], din["sut"][:])
        ident_sb = sb.tile([128, 128], F32)
        nc.sync.dma_start(ident_sb[:], din["ident"][:])
        onescol_sb = sb.tile([128, 1], F32)
        nc.sync.dma_start(onescol_sb[:], din["ones_col"][:])
        onesrow_sb = sb.tile([1, 128], F32)
        nc.sync.dma_start(onesrow_sb[:], din["ones_row"][:])
        iota_t_sb = sb.tile([128, NS], F32)
        nc.sync.dma_start(iota_t_sb[:], din["iota_t"][:])
        tloc_sb = sb.tile([128, NS], F32)
        nc.sync.dma_start(tloc_sb[:], din["tloc"][:])
        own96_sb = sb.tile([128, NS], F32)
        nc.sync.dma_start(own96_sb[:], din["own96"][:])
        iota_c_sb = sb.tile([128, C], F32)
        nc.sync.dma_start(iota_c_sb[:], din["iota_c"][:])
        onehot_sb = sb.tile([128, E], F32)
        nc.sync.dma_start(onehot_sb[:], din["onehot_e"][:])
        gate_b_sb = sb.tile([128, E], F32)
        nc.sync.dma_start(gate_b_sb[:], din["gate_b_rep"][:])
        b1_sb = sb.tile([128, H // 128], F32)
        nc.sync.dma_start(b1_sb[:], din["b1_e"][:])
        b2_sb = sb.tile([128, D], F32)
        nc.sync.dma_start(b2_sb[:], din["b2_rep"][:])

        # zero-init the a2a input; pad rows carry t_local=OOB so owners skip them
        # (concurrent RMW adds to one row within a single indirect DMA race).
        a2a_in = dram.tile([AROWS, AW], BF16)
        a2a_out = dram.tile([AROWS, AW], BF16)
        zero_bf = sb.tile([128, AROWS // 128, AW], BF16)
        nc.vector.memset(zero_bf[:], 0.0)
        nc.vector.memset(zero_bf[:, :, 1024:1025], OOB)
        nc.sync.dma_start(a2a_in[:].rearrange("(g p) w -> p g w", p=128), zero_bf[:])

        # zero-init the output slice (we scatter-add into it at the end)
        zero_f = sb.tile([128, TS // 128, D], F32)
        nc.vector.memset(zero_f[:], 0.0)
        nc.sync.dma_start(out_ext[:].rearrange("(g p) d1 -> p g d1", p=128), zero_f[:])

        # ---------------- phase 1: gate on own 256-token slice ----------------
        ag_in = dram.tile([TS, E], F32)
        ag_out = dram.tile([T, E], F32)
        logits_sb = sb.tile([128, NS, E], F32)

        with tc.tile_pool(name="gate_sb", bufs=1) as gsb, \
             tc.tile_pool(name="gate_ps", bufs=1, space="PSUM") as gps:
            xs_sb = gsb.tile([128, 2, D], F32)
            nc.sync.dma_start(xs_sb[:], din["x_slice"][:].rearrange("(m p) d1 -> p m d1", p=128))
            gw_sb = gsb.tile([128, D // 128, E], F32)
            nc.sync.dma_start(gw_sb[:], din["gate_w"][:].rearrange("(ko ki) e -> ki ko e", ki=128))

            xT = gsb.tile([128, D // 128, TS], F32)
            for m in range(2):
                for dch in range(D // 128):
                    tp = gps.tile([128, 128], F32, tag="tr")
                    nc.tensor.transpose(tp[:], xs_sb[:, m, dch * 128:(dch + 1) * 128], ident_sb[:])
                    nc.vector.tensor_copy(xT[:, dch, m * 128:(m + 1) * 128], tp[:])

            logit_sl = gsb.tile([128, 2, E], F32)
            for m in range(2):
                gp = gps.tile([128, E], F32, tag="gmm")
                for dch in range(D // 128):
                    nc.tensor.matmul(gp[:], lhsT=xT[:, dch, m * 128:(m + 1) * 128],
                                     rhs=gw_sb[:, dch, :],
                                     start=(dch == 0), stop=(dch == D // 128 - 1))
                nc.vector.tensor_tensor(logit_sl[:, m, :], gp[:], gate_b_sb[:], op=OP.add)
            nc.sync.dma_start(ag_in[:].rearrange("(m p) e -> p m e", p=128), logit_sl[:])

        nc.gpsimd.collective_compute(
            "AllGather", OP.bypass, replica_groups=[list(range(E))],
            ins=[ag_in[:].opt()], outs=[ag_out[:].opt()],
        )
        nc.sync.dma_start(logits_sb[:], ag_out[:].rearrange("(s p) e -> p s e", p=128))

        def diag(ap, rows, cols):
            # write an SBUF tile into out_ext[0:rows, 0:cols] for debugging
            nc.sync.dma_start(out_ext[0:rows, 0:cols], ap)

        if PHASE <= 1:
            diag(logits_sb[:].rearrange("p s e -> p (s e)"), 128, NS * E)
            ctx.close()
            nc.finalize()
            return nc

        # ---------------- phase 2: top-2 routing (identical on all cores) ----------------
        rsb = ctx.enter_context(tc.tile_pool(name="route_sb", bufs=1))

        m1 = rsb.tile([128, NS], F32)
        nc.vector.reduce_max(m1[:], logits_sb[:], axis=mybir.AxisListType.X)
        is1 = rsb.tile([128, NS, E], F32)
        nc.vector.tensor_tensor(is1[:], logits_sb[:], m1[:, :, None].to_broadcast([128, NS, E]), op=OP.is_equal)
        lneg = rsb.tile([128, NS, E], F32)
        nc.vector.tensor_scalar_mul(lneg[:], is1[:], -BIGF)
        nc.vector.tensor_tensor(lneg[:], logits_sb[:], lneg[:], op=OP.add)
        m2 = rsb.tile([128, NS], F32)
        nc.vector.reduce_max(m2[:], lneg[:], axis=mybir.AxisListType.X)
        is2 = rsb.tile([128, NS, E], F32)
        nc.vector.tensor_tensor(is2[:], lneg[:], m2[:, :, None].to_broadcast([128, NS, E]), op=OP.is_equal)
        d21 = rsb.tile([128, NS], F32)
        nc.vector.tensor_tensor(d21[:], m2[:], m1[:], op=OP.subtract)
        wB = rsb.tile([128, NS], F32)
        nc.scalar.activation(wB[:], d21[:], AF.Sigmoid)
        wA = rsb.tile([128, NS], F32)
        nc.vector.tensor_scalar(wA[:], wB[:], -1.0, 1.0, op0=OP.mult, op1=OP.add)

        mask_all = rsb.tile([128, NS, E], F32)
        nc.vector.tensor_tensor(mask_all[:], is1[:], is2[:], op=OP.add)
        g_all = rsb.tile([128, NS, E], F32)
        t1 = rsb.tile([128, NS, E], F32)
        nc.vector.tensor_tensor(t1[:], is1[:], wA[:, :, None].to_broadcast([128, NS, E]), op=OP.mult)
        nc.vector.tensor_tensor(g_all[:], is2[:], wB[:, :, None].to_broadcast([128, NS, E]), op=OP.mult)
        nc.vector.tensor_tensor(g_all[:], g_all[:], t1[:], op=OP.add)

        # cumulative slots over token order (t = s*128 + p), all experts at once
        mask_f = mask_all[:].rearrange("p s e -> p (s e)")
        with tc.tile_pool(name="cum_ps", bufs=1, space="PSUM") as cps:
            e1p = cps.tile([128, NS * E], F32, tag="e1")
            nc.tensor.matmul(e1p[:], lhsT=sut_sb[:], rhs=mask_f, start=True, stop=True)
            E1 = rsb.tile([128, NS, E], F32)
            nc.vector.tensor_copy(E1[:].rearrange("p s e -> p (s e)"), e1p[:])

            totp = cps.tile([1, NS * E], F32, tag="tot")
            nc.tensor.matmul(totp[:], lhsT=onescol_sb[:], rhs=mask_f, start=True, stop=True)
            tot = rsb.tile([1, NS, E], F32)
            nc.vector.tensor_copy(tot[:].rearrange("p s e -> p (s e)"), totp[:])

            # co_both[0] = global exclusive scan over s; co_both[1] = per-owner reset
            co_a = rsb.tile([1, NS, E], F32)
            co_b = rsb.tile([1, NS, E], F32)
            nc.vector.memset(co_a[:], 0.0)
            nc.vector.tensor_copy(co_a[:, 1:NS, :], tot[:, 0:NS - 1, :])
            src, dst = co_a, co_b
            for k in (1, 2, 4, 8):
                nc.vector.tensor_copy(dst[:], src[:])
                nc.vector.tensor_tensor(dst[:, k:NS, :], src[:, k:NS, :], src[:, 0:NS - k, :], op=OP.add)
                src, dst = dst, src
            co_g = src  # [1, NS, E] exclusive prefix of tot over s
            co_both = rsb.tile([1, 2, NS, E], F32)
            nc.vector.tensor_copy(co_both[:, 0, :, :], co_g[:, :, :])
            nc.vector.memset(co_both[:, 1, :, :], 0.0)
            nc.vector.tensor_copy(co_both[:, 1, 1:NS:2, :], tot[:, 0:NS:2, :])

            bcp = cps.tile([128, 2 * NS * E], F32, tag="bc")
            nc.tensor.matmul(bcp[:], lhsT=onesrow_sb[:], rhs=co_both[:].rearrange("p a s e -> p (a s e)"),
                             start=True, stop=True)
            cob = rsb.tile([128, 2, NS, E], F32)
            nc.vector.tensor_copy(cob[:].rearrange("p a s e -> p (a s e)"), bcp[:])

        slot_g = rsb.tile([128, NS, E], F32)
        nc.vector.tensor_tensor(slot_g[:], E1[:], cob[:, 0], op=OP.add)
        r_own = rsb.tile([128, NS, E], F32)
        nc.vector.tensor_tensor(r_own[:], E1[:], cob[:, 1], op=OP.add)
        slotp = rsb.tile([128, NS, E], F32)
        nc.vector.tensor_tensor(slotp[:], r_own[:], own96_sb[:, :, None].to_broadcast([128, NS, E]), op=OP.add)
        # overflow guard: r_own >= PAIRC would corrupt the next owner block -> push OOB
        ovf = rsb.tile([128, NS, E], F32)
        nc.vector.tensor_scalar(ovf[:], r_own[:], float(PAIRC), 2.0 * OOB, op0=OP.is_ge, op1=OP.mult)
        nc.vector.tensor_tensor(slotp[:], slotp[:], ovf[:], op=OP.add)

        if PHASE <= 2:
            diag(slot_g[:].rearrange("p s e -> p (s e)"), 128, NS * E)
            ctx.close()
            nc.finalize()
            return nc

        # ---------------- phase 3: this core's expert columns ----------------
        def extract(dst, src3):
            tmp = rsb.tile([128, NS, E], F32, tag="exttmp")
            nc.vector.tensor_tensor(tmp[:], src3[:], onehot_sb[:, None, :].to_broadcast([128, NS, E]), op=OP.mult)
            nc.vector.reduce_sum(dst[:], tmp[:], axis=mybir.AxisListType.X)

        m_e = rsb.tile([128, NS], F32)
        extract(m_e, mask_all)
        slot_e = rsb.tile([128, NS], F32)
        extract(slot_e, slot_g)
        slotp_e = rsb.tile([128, NS], F32)
        extract(slotp_e, slotp)
        g_e = rsb.tile([128, NS], F32)
        extract(g_e, g_all)

        # rv columns: [token id, OOB - slotp, gate weight, t_local]
        rv = rsb.tile([128, NS, 4], F32)
        nc.vector.tensor_copy(rv[:, :, 0:1], iota_t_sb[:, :, None])
        nc.vector.tensor_scalar(rv[:, :, 1:2], slotp_e[:, :, None], -1.0, OOB, op0=OP.mult, op1=OP.add)
        nc.vector.tensor_copy(rv[:, :, 2:3], g_e[:, :, None])
        nc.vector.tensor_copy(rv[:, :, 3:4], tloc_sb[:, :, None])

        # slot_e masked: non-routed tokens -> -1 (never matches iota_c)
        sm = rsb.tile([128, NS], F32)
        nc.vector.tensor_scalar_add(sm[:], slot_e[:], 1.0)
        nc.vector.tensor_tensor(sm[:], sm[:], m_e[:], op=OP.mult)
        nc.vector.tensor_scalar_add(sm[:], sm[:], -1.0)

        idx_i = rsb.tile([128, NC_CHUNKS], I32)
        slotp_i = rsb.tile([128, NC_CHUNKS], I32)
        g_c = rsb.tile([128, NC_CHUNKS], F32)
        tloc_c = rsb.tile([128, NC_CHUNKS], F32)
        nc.vector.memset(idx_i[:], 0)
        nc.vector.memset(slotp_i[:], int(OOB))
        nc.vector.memset(g_c[:], 0.0)
        nc.vector.memset(tloc_c[:], 0.0)

        with tc.tile_pool(name="s_sb", bufs=NS) as ssb, \
             tc.tile_pool(name="ext_ps", bufs=1, space="PSUM") as eps:
            eps_tiles = [eps.tile([128, 4], F32, tag=f"ext{mc}", name=f"ext{mc}")
                         for mc in range(NC_CHUNKS)]
            S_list = []
            for s in range(NS):
                S_s = ssb.tile([128, C], F32, tag="S")
                nc.vector.tensor_tensor(S_s[:], sm[:, s:s + 1].to_broadcast([128, C]), iota_c_sb[:], op=OP.is_equal)
                S_list.append(S_s)
            for mc in range(NC_CHUNKS):
                mw = min(128, C - mc * 128)
                for s in range(NS):
                    nc.tensor.matmul(eps_tiles[mc][:mw], lhsT=S_list[s][:, mc * 128: mc * 128 + mw],
                                     rhs=rv[:, s:s + 1, :], start=(s == 0), stop=(s == NS - 1))
            fext = rsb.tile([128, NC_CHUNKS], F32, name="fext")
            for mc in range(NC_CHUNKS):
                mw = min(128, C - mc * 128)
                ep = eps_tiles[mc]
                nc.vector.tensor_copy(fext[:mw, mc:mc + 1], ep[:mw, 0:1])
                nc.vector.tensor_copy(idx_i[:mw, mc:mc + 1], ep[:mw, 0:1])
                ng = rsb.tile([128, 1], F32, tag="ng")
                nc.vector.tensor_scalar(ng[:mw], ep[:mw, 1:2], -1.0, OOB, op0=OP.mult, op1=OP.add)
                nc.vector.tensor_copy(slotp_i[:mw, mc:mc + 1], ng[:mw, :])
                nc.vector.tensor_copy(g_c[:mw, mc:mc + 1], ep[:mw, 2:3])
                nc.vector.tensor_copy(tloc_c[:mw, mc:mc + 1], ep[:mw, 3:4])

        if PHASE <= 3:
            fidx = rsb.tile([128, NC_CHUNKS], F32, name="fidx")
            fsl = rsb.tile([128, NC_CHUNKS], F32, name="fsl")
            nc.vector.tensor_copy(fidx[:], idx_i[:])
            nc.vector.tensor_copy(fsl[:], slotp_i[:])
            nc.sync.dma_start(out_ext[0:128, 0:NC_CHUNKS], fidx[:])
            nc.sync.dma_start(out_ext[0:128, 16:16 + NC_CHUNKS], fsl[:])
            nc.sync.dma_start(out_ext[0:128, 32:32 + NC_CHUNKS], g_c[:])
            nc.sync.dma_start(out_ext[0:128, 48:48 + NC_CHUNKS], tloc_c[:])
            nc.sync.dma_start(out_ext[128:256, 0:NS], sm[:])
            nc.sync.dma_start(out_ext[128:256, 16:16 + NS], m_e[:])
            nc.sync.dma_start(out_ext[128:256, 32:32 + NS], g_e[:])
            nc.sync.dma_start(out_ext[128:256, 48:48 + NS], slotp_e[:])
            nc.sync.dma_start(out_ext[128:256, 64:64 + NS * 4], rv[:].rearrange("p s k -> p (s k)"))
            nc.sync.dma_start(out_ext[0:128, 80:80 + NC_CHUNKS], fext[:])
            ctx.close()
            nc.finalize()
            return nc

        # ---------------- phase 4: gather x rows + transpose ----------------
        xgT = sb.tile([128, D // 128, NC_CHUNKS * 128], F32R)
        with tc.tile_pool(name="xg_sb", bufs=1) as xsb, \
             tc.tile_pool(name="tr_ps", bufs=2, space="PSUM") as tps:
            xg = xsb.tile([128, NC_CHUNKS, D], F32)
            for mc in range(NC_CHUNKS):
                nc.gpsimd.indirect_dma_start(
                    out=xg[:, mc, :], out_offset=None,
                    in_=din["x_full"][:],
                    in_offset=bass.IndirectOffsetOnAxis(ap=idx_i[:, mc:mc + 1], axis=0),
                )
            for mc in range(NC_CHUNKS):
                for dch in range(D // 128):
                    tp = tps.tile([128, 128], F32, tag="xtr")
                    nc.tensor.transpose(tp[:], xg[:, mc, dch * 128:(dch + 1) * 128], ident_sb[:])
                    nc.vector.tensor_copy(xgT[:, dch, mc * 128:(mc + 1) * 128], tp[:])

        if PHASE <= 4:
            fxg = sb.tile([128, 576], F32, name="fxg")
            nc.vector.tensor_copy(fxg[:], xgT[:, 0, 0:576])
            diag(fxg[:], 128, 576)
            ctx.close()
            nc.finalize()
            return nc

        # ---------------- phase 5: MM1  hT = gelu(w1^T x^T + b1)  [H, C] ----------------
        hT = sb.tile([128, H // 128, C], F32R)
        with tc.tile_pool(name="w1_sb", bufs=2) as w1p, \
             tc.tile_pool(name="mm1_ps", bufs=3, space="PSUM") as m1ps:
            for hb in range(H // 512):
                w1blk = w1p.tile([128, D // 128, 512], F32R, tag="w1")
                nc.gpsimd.dma_start(
                    w1blk[:],
                    din["w1_e"][:].rearrange("(ko ki) h -> ki ko h", ki=128)[:, :, hb * 512:(hb + 1) * 512])
                for hsub in range(4):
                    hc = hb * 4 + hsub
                    for nh in range(2):
                        php = m1ps.tile([128, CH], F32, tag="mm1")
                        for k in range(D // 128):
                            nc.tensor.matmul(php[:], lhsT=w1blk[:, k, hsub * 128:(hsub + 1) * 128],
                                             rhs=xgT[:, k, nh * CH:(nh + 1) * CH],
                                             start=(k == 0), stop=(k == D // 128 - 1))
                        nc.scalar.activation(hT[:, hc, nh * CH:(nh + 1) * CH], php[:],
                                             AF.Gelu, bias=b1_sb[:, hc:hc + 1])

        if PHASE <= 5:
            fh = sb.tile([128, 576], F32, name="fh")
            nc.vector.tensor_copy(fh[:], hT[:, 0, 0:576])
            diag(fh[:], 128, 576)
            ctx.close()
            nc.finalize()
            return nc

        # ---------------- phase 6: MM2  y = hT^T w2 + b2, weight, cast ----------------
        y_bf = sb.tile([128, NC_CHUNKS, AW], BF16)
        for mc in range(NC_CHUNKS):
            mw = min(128, C - mc * 128)
            nc.vector.tensor_copy(y_bf[:mw, mc, 1024:1025], tloc_c[:mw, mc:mc + 1])
        with tc.tile_pool(name="w2_sb", bufs=3) as w2p, \
             tc.tile_pool(name="mm2_ps", bufs=1, space="PSUM") as m2ps, \
             tc.tile_pool(name="ytmp_sb", bufs=2) as ytp:
            for dh in range(2):
                psums = []
                for mc in range(NC_CHUNKS):
                    mw = min(128, C - mc * 128)
                    psums.append(m2ps.tile([128, 512], F32, tag=f"mm2_{mc}", name=f"mm2_{mc}"))
                for kb in range(H // 512):
                    w2blk = w2p.tile([128, 4, 512], F32R, tag="w2")
                    nc.gpsimd.dma_start(
                        w2blk[:],
                        din["w2_e"][:].rearrange("(ko ki) d1 -> ki ko d1", ki=128)
                        [:, kb * 4:(kb + 1) * 4, dh * 512:(dh + 1) * 512])
                    for ks in range(4):
                        k = kb * 4 + ks
                        for mc in range(NC_CHUNKS):
                            mw = min(128, C - mc * 128)
                            nc.tensor.matmul(psums[mc][:mw], lhsT=hT[:, k, mc * 128: mc * 128 + mw],
                                             rhs=w2blk[:, ks, :],
                                             start=(k == 0), stop=(k == H // 128 - 1))
                for mc in range(NC_CHUNKS):
                    mw = min(128, C - mc * 128)
                    yt = ytp.tile([128, 512], F32, tag="yt")
                    nc.vector.tensor_tensor(yt[:mw], psums[mc][:mw], b2_sb[:mw, dh * 512:(dh + 1) * 512], op=OP.add)
                    nc.vector.tensor_scalar_mul(y_bf[:mw, mc, dh * 512:(dh + 1) * 512], yt[:mw], g_c[:mw, mc:mc + 1])

        if PHASE <= 6:
            fy = sb.tile([128, NC_CHUNKS, D], F32, name="fy")
            nc.vector.tensor_copy(fy[:], y_bf[:, :, 0:D])
            diag(fy[:, 0, :], 128, D)
            ctx.close()
            nc.finalize()
            return nc

        # ---------------- phase 7: scatter to a2a_in, AllToAll ----------------
        for mc in range(NC_CHUNKS):
            mw = min(128, C - mc * 128)
            nc.gpsimd.indirect_dma_start(
                out=a2a_in[:], out_offset=bass.IndirectOffsetOnAxis(ap=slotp_i[:mw, mc:mc + 1], axis=0),
                in_=y_bf[:mw, mc, :], in_offset=None,
                bounds_check=AROWS - 1, oob_is_err=False,
            )
        nc.gpsimd.collective_compute(
            "AllToAll", OP.bypass, replica_groups=[list(range(E))],
            ins=[a2a_in[:].opt()], outs=[a2a_out[:].opt()],
        )

        if PHASE <= 7:
            fa = sb.tile([128, 6, AW], F32, name="fa")
            nc.gpsimd.dma_start(fa[:], a2a_out[:].rearrange("(g p) w -> p g w", p=128))
            diag(fa[:, 0, 0:D], 128, D)
            ctx.close()
            nc.finalize()
            return nc

        # ---------------- phase 8: owner combine ----------------
        with tc.tile_pool(name="own_sb", bufs=1) as osb:
            own_bf = osb.tile([PAIRC, E, AW], BF16)
            nc.sync.dma_start(own_bf[:], a2a_out[:].rearrange("(e r) w -> r e w", r=PAIRC))
            y32 = osb.tile([PAIRC, E, D], F32)
            nc.vector.tensor_copy(y32[:], own_bf[:, :, 0:D])
            tsend = osb.tile([PAIRC, E], I32)
            nc.vector.tensor_copy(tsend[:], own_bf[:, :, D:D + 1])
            for e in range(E):
                nc.gpsimd.indirect_dma_start(
                    out=out_ext[:], out_offset=bass.IndirectOffsetOnAxis(ap=tsend[:, e:e + 1], axis=0),
                    in_=y32[:, e, :], in_offset=None,
                    bounds_check=TS - 1, oob_is_err=False,
                    compute_op=OP.add,
                )

    nc.finalize()
    return nc


def _host_inputs(x, gate_w, gate_b, w1, b1, w2, b2):
    xf = np.ascontiguousarray(x.reshape(T, D), dtype=np.float32)
    p = np.arange(128, dtype=np.float32)[:, None]
    s = np.arange(NS, dtype=np.float32)[None, :]
    iota_t = (s * 128 + p).astype(np.float32)
    tloc = ((s % 2) * 128 + p).astype(np.float32)
    own96 = np.broadcast_to(np.floor(s / 2) * PAIRC, (128, NS)).astype(np.float32)
    iota_c = np.broadcast_to(np.arange(C, dtype=np.float32), (128, C)).copy()
    sut = np.triu(np.ones((128, 128), np.float32), k=1)
    ident = np.eye(128, dtype=np.float32)
    ones_col = np.ones((128, 1), np.float32)
    ones_row = np.ones((1, 128), np.float32)
    gate_b_rep = np.broadcast_to(gate_b.astype(np.float32), (128, E)).copy()

    in_maps = []
    for c in range(E):
        in_maps.append({
            "x_full": xf,
            "x_slice": np.ascontiguousarray(xf[c * TS:(c + 1) * TS]),
            "gate_w": np.ascontiguousarray(gate_w, dtype=np.float32),
            "gate_b_rep": gate_b_rep,
            "w1_e": np.ascontiguousarray(w1[c], dtype=np.float32),
            "b1_e": np.ascontiguousarray(b1[c].reshape(H // 128, 128).T),
            "w2_e": np.ascontiguousarray(w2[c], dtype=np.float32),
            "b2_rep": np.broadcast_to(b2[c].astype(np.float32), (128, D)).copy(),
            "onehot_e": np.broadcast_to(np.eye(E, dtype=np.float32)[c], (128, E)).copy(),
            "ident": ident,
            "sut": sut,
            "ones_col": ones_col,
            "ones_row": ones_row,
            "iota_t": iota_t,
            "tloc": tloc,
            "own96": own96,
            "iota_c": iota_c,
        })
    return in_maps


def kernel(x, gate_w, gate_b, w1, b1, w2, b2):
    in_maps = _host_inputs(np.asarray(x), np.asarray(gate_w), np.asarray(gate_b),
                           np.asarray(w1), np.asarray(b1), np.asarray(w2), np.asarray(b2))
    nc = _build_nc()
    trace = bool(int(os.environ.get("BASSMOE_TRACE", "0")))
    res = run_bass_kernel_spmd(nc, in_maps, core_ids=list(range(E)), trace=trace,
                               trace_cores=list(range(E)) if trace else None)
    LAST_EXEC_NS[0] = res.exec_time_ns
    out = np.concatenate([res.results[c]["out"] for c in range(E)], axis=0)
    return out.reshape(1, T, D).astype(np.float32)
